# revision 6
# baseline (speedup 1.0000x reference)
"""Trainium2 Bass kernel for retrieval-knn one-hot accuracy preamble.

Computation (reference semantics):
    sim = y_pred @ image_features.T          # [B=512, N=100000] cosine ranking
    top10 indices per row -> labels y[idx] -> one-hot [B, C=1854]

Notes on the implementation:
  * Query normalization is skipped: it scales each sim row by a positive
    constant and cannot change the per-row ranking.
  * The matmul runs as a bf16 hi/lo split (hi*hi + hi*lo + lo*hi) which
    reproduces fp32-level precision at 3x bf16 throughput. The split and the
    feature transpose (contraction dim on partitions) happen on the host as
    part of input sharding/layout.
  * Work is sharded across 8 NeuronCores by query batch (64 queries/core,
    gallery replicated) - fully data-parallel, no collectives.
  * Per core, gallery is streamed in 2048-column super-chunks. Matmul output
    pairs are stacked on 128 partitions (rows q / q+64 hold different gallery
    halves) so the vector-engine top-k screen runs at full width.
  * Screen: per (query, 1024-gallery-block) top-16 via max8/max_index/
    match_replace (sound: any global top-10 element is top-10 within its
    block). Final: top-10 over the 1568 candidates, index resolution via
    indirect DMA gathers, one-hot built by iota-compare.
"""

import numpy as np
import ml_dtypes

BF16 = ml_dtypes.bfloat16

B, N_GAL, D, C, TOPK = 512, 100000, 512, 1854, 10
NCORES = 8
M = B // NCORES            # queries per core
SUPER = 2048               # gallery columns per super-chunk
NSUPER_FULL = 49           # 49 * 2048 = 100352 >= 100000
NEG_SENTINEL = -1e30


def _build(nsuper, m=M, c=C, debug=False):
    """Build the Bass program for one core (SPMD across all cores)."""
    import concourse.bass as bass
    import concourse.mybir as mybir
    import concourse.tile as tile
    from concourse import bacc

    npad = nsuper * SUPER
    cand = nsuper * 16          # candidates per partition row
    candq = 2 * cand            # candidates per query after restack
    f32 = mybir.dt.float32
    bf16 = mybir.dt.bfloat16
    u32 = mybir.dt.uint32
    i32 = mybir.dt.int32
    Add = mybir.AluOpType.add
    Max = mybir.AluOpType.max
    IsEq = mybir.AluOpType.is_equal

    nc = bacc.Bacc("TRN2", target_bir_lowering=False, debug=False)

    with tile.TileContext(nc) as tc:
        with (
            tc.tile_pool(name="dram", bufs=1, space="DRAM") as dram,
            tc.tile_pool(name="const", bufs=1) as constp,
            tc.tile_pool(name="stream", bufs=3) as streamp,
            tc.tile_pool(name="psum", bufs=2, space="PSUM") as psump,
            tc.tile_pool(name="work", bufs=3) as workp,
            tc.tile_pool(name="fin", bufs=1) as finp,
        ):
            # ---- kernel I/O ----
            fhi = dram.tile([4, 128, npad], bf16, kind="ExternalInput",
                            name="fhi", uniquify=False)
            flo = dram.tile([4, 128, npad], bf16, kind="ExternalInput",
                            name="flo", uniquify=False)
            yphi = dram.tile([4, 128, m], bf16, kind="ExternalInput",
                             name="yphi", uniquify=False)
            yplo = dram.tile([4, 128, m], bf16, kind="ExternalInput",
                             name="yplo", uniquify=False)
            ylab = dram.tile([npad, 1], i32, kind="ExternalInput",
                             name="ylab", uniquify=False)
            corr = dram.tile([128, cand], u32, kind="ExternalInput",
                             name="corr", uniquify=False)
            qbase = dram.tile([m, TOPK], u32, kind="ExternalInput",
                              name="qbase", uniquify=False)
            iotac = dram.tile([m, c], f32, kind="ExternalInput",
                              name="iotac", uniquify=False)
            out = dram.tile([m, c], f32, kind="ExternalOutput",
                            name="out", uniquify=False)
            # DRAM scratch for candidate-index resolution gathers
            cidx_dram = dram.tile([m * candq, 1], u32, kind="Internal",
                                  name="cidx_scratch", uniquify=False)

            # ---- persistent SBUF ----
            yph_sb = constp.tile([128, 4, m], bf16)
            ypl_sb = constp.tile([128, 4, m], bf16)
            corr_sb = constp.tile([128, cand], u32)
            qb_sb = constp.tile([m, TOPK], u32)
            iota_sb = constp.tile([m, c], f32)
            candval = constp.tile([128, cand], f32)
            candidx = constp.tile([128, cand], u32)

            nc.sync.dma_start(out=yph_sb[:, :, :],
                              in_=yphi[:, :, :].rearrange("k p m -> p k m"))
            nc.sync.dma_start(out=ypl_sb[:, :, :],
                              in_=yplo[:, :, :].rearrange("k p m -> p k m"))
            nc.sync.dma_start(out=corr_sb[:, :], in_=corr[:, :])
            nc.sync.dma_start(out=qb_sb[:, :], in_=qbase[:, :])
            nc.sync.dma_start(out=iota_sb[:, :], in_=iotac[:, :])

            # ---- main streaming loop ----
            for s in range(nsuper):
                g0 = s * SUPER
                fhi_sb = streamp.tile([128, 4, SUPER], bf16, tag="fhi")
                flo_sb = streamp.tile([128, 4, SUPER], bf16, tag="flo")
                nc.sync.dma_start(
                    out=fhi_sb[:, :, :],
                    in_=fhi[:, :, g0:g0 + SUPER].rearrange("k p g -> p k g"))
                nc.sync.dma_start(
                    out=flo_sb[:, :, :],
                    in_=flo[:, :, g0:g0 + SUPER].rearrange("k p g -> p k g"))

                # chunk ci covers gallery [g0 + ci*512, g0 + (ci+1)*512).
                # Each chunk accumulates in its own PSUM bank; chunks 2/3
                # write partitions 64:128 so the screen below can run at
                # full 128-partition width. A bank's unused half is wasted
                # rather than shared - two accumulation groups in one bank
                # would break has_written semantics.
                ps0 = psump.tile([128, 512], f32, tag="ps0")
                ps1 = psump.tile([128, 512], f32, tag="ps1")
                ps2 = psump.tile([128, 512], f32, tag="ps2")
                ps3 = psump.tile([128, 512], f32, tag="ps3")
                chunk_out = [
                    ps0[0:m, :], ps1[0:m, :],
                    ps2[m:2 * m, :], ps3[m:2 * m, :],
                ]
                terms = [(yph_sb, fhi_sb), (yph_sb, flo_sb), (ypl_sb, fhi_sb)]
                for k in range(4):
                    for t, (lw, rs) in enumerate(terms):
                        first = (k == 0 and t == 0)
                        last = (k == 3 and t == len(terms) - 1)
                        for ci in range(4):
                            nc.tensor.matmul(
                                chunk_out[ci],
                                lhsT=lw[:, k, :],
                                rhs=rs[:, k, ci * 512:(ci + 1) * 512],
                                start=first, stop=last)

                # stack into [128, 1024] SBUF (ACT engine copies)
                sview = workp.tile([128, 1024], f32, tag="sview")
                nc.scalar.copy(out=sview[0:m, 0:512], in_=ps0[0:m, :])
                nc.scalar.copy(out=sview[0:m, 512:1024], in_=ps1[0:m, :])
                nc.scalar.copy(out=sview[m:2 * m, 0:512], in_=ps2[m:2 * m, :])
                nc.scalar.copy(out=sview[m:2 * m, 512:1024],
                               in_=ps3[m:2 * m, :])

                # per-row top-16 screen
                cv0 = candval[:, s * 16:s * 16 + 8]
                cv1 = candval[:, s * 16 + 8:s * 16 + 16]
                ci0 = candidx[:, s * 16:s * 16 + 8]
                ci1 = candidx[:, s * 16 + 8:s * 16 + 16]
                svrep = workp.tile([128, 1024], f32, tag="svrep")
                nc.vector.max(out=cv0, in_=sview[:, :])
                nc.vector.max_index(out=ci0, in_max=cv0, in_values=sview[:, :])
                nc.vector.match_replace(out=svrep[:, :], in_to_replace=cv0,
                                        in_values=sview[:, :],
                                        imm_value=NEG_SENTINEL)
                nc.vector.max(out=cv1, in_=svrep[:, :])
                nc.vector.max_index(out=ci1, in_max=cv1, in_values=svrep[:, :])

            # ---- candidate finalization ----
            # local -> global gallery indices
            nc.vector.tensor_tensor(out=candidx[:, :], in0=candidx[:, :],
                                    in1=corr_sb[:, :], op=Add)
            # restack [128, cand] -> [m, 2*cand] (row q+64 appended to row q)
            finval = finp.tile([m, candq], f32)
            nc.sync.dma_start(out=finval[:, 0:cand], in_=candval[0:m, :])
            nc.sync.dma_start(out=finval[:, cand:candq],
                              in_=candval[m:2 * m, :])
            cidx2d = cidx_dram[:, :].rearrange("(q j) one -> q (j one)", q=m)
            nc.sync.dma_start(out=cidx2d[:, 0:cand], in_=candidx[0:m, :])
            nc.sync.dma_start(out=cidx2d[:, cand:candq],
                              in_=candidx[m:2 * m, :])

            # final top-10 positions among the candidates
            v1 = workp.tile([m, 8], f32, tag="v8")
            v2 = workp.tile([m, 8], f32, tag="v8b")
            p1 = workp.tile([m, 8], u32, tag="p8")
            p2 = workp.tile([m, 8], u32, tag="p8b")
            finrep = finp.tile([m, candq], f32)
            nc.vector.max(out=v1[:, :], in_=finval[:, :])
            nc.vector.max_index(out=p1[:, :], in_max=v1[:, :],
                                in_values=finval[:, :])
            nc.vector.match_replace(out=finrep[:, :], in_to_replace=v1[:, :],
                                    in_values=finval[:, :],
                                    imm_value=NEG_SENTINEL)
            nc.vector.max(out=v2[:, :], in_=finrep[:, :])
            nc.vector.max_index(out=p2[:, :], in_max=v2[:, :],
                                in_values=finrep[:, :])

            pos = workp.tile([m, TOPK], u32, tag="pos")
            nc.vector.tensor_copy(out=pos[:, 0:8], in_=p1[:, :])
            nc.vector.tensor_copy(out=pos[:, 8:TOPK], in_=p2[:, 0:TOPK - 8])
            offs = workp.tile([m, TOPK], u32, tag="offs")
            nc.vector.tensor_tensor(out=offs[:, :], in0=pos[:, :],
                                    in1=qb_sb[:, :], op=Add)

            # gather global gallery indices, then labels
            # (one indirect DMA per top-k slot: [m, 1] offsets per partition
            # is the layout the runtime's unroller supports)
            gidx = workp.tile([m, TOPK], u32, tag="gidx")
            for kk in range(TOPK):
                nc.gpsimd.indirect_dma_start(
                    out=gidx[:, kk:kk + 1], out_offset=None,
                    in_=cidx_dram[:, :],
                    in_offset=bass.IndirectOffsetOnAxis(
                        ap=offs[:, kk:kk + 1], axis=0))
            labs = workp.tile([m, TOPK], i32, tag="labs")
            for kk in range(TOPK):
                nc.gpsimd.indirect_dma_start(
                    out=labs[:, kk:kk + 1], out_offset=None,
                    in_=ylab[:, :],
                    in_offset=bass.IndirectOffsetOnAxis(
                        ap=gidx[:, kk:kk + 1], axis=0))
            labf = workp.tile([m, TOPK], f32, tag="labf")
            nc.vector.tensor_copy(out=labf[:, :], in_=labs[:, :])

            # one-hot via iota-compare
            acc = finp.tile([m, c], f32)
            tmp = workp.tile([m, c], f32, tag="onehot")
            nc.vector.tensor_scalar(acc[:, :], iota_sb[:, :], labf[:, 0:1],
                                    None, op0=IsEq)
            for kk in range(1, TOPK):
                nc.vector.tensor_scalar(tmp[:, :], iota_sb[:, :],
                                        labf[:, kk:kk + 1], None, op0=IsEq)
                nc.vector.tensor_tensor(out=acc[:, :], in0=acc[:, :],
                                        in1=tmp[:, :], op=Max)
            nc.sync.dma_start(out=out[:, :], in_=acc[:, :])

            if debug:
                dbg_specs = [
                    ("dbg_candval", candval, f32),
                    ("dbg_candidx", candidx, u32),
                    ("dbg_finval", finval, f32),
                    ("dbg_finrep", finrep, f32),
                    ("dbg_pos", pos, u32),
                    ("dbg_offs", offs, u32),
                    ("dbg_gidx", gidx, u32),
                    ("dbg_labs", labs, i32),
                    ("dbg_v1", v1, f32),
                    ("dbg_p1", p1, u32),
                ]
                for dname, src, dt_ in dbg_specs:
                    dtile = dram.tile(list(src.shape), dt_,
                                      kind="ExternalOutput", name=dname,
                                      uniquify=False)
                    nc.sync.dma_start(out=dtile[...], in_=src[...])

    nc.compile()
    return nc


def _split_bf16(x):
    hi = x.astype(BF16)
    lo = (x - hi.astype(np.float32)).astype(BF16)
    return hi, lo


def _prep_inputs(y_pred, feats, y, nsuper, ncores=NCORES):
    """Host-side sharding/layout: pad + transpose gallery, bf16 hi/lo split,
    per-core query slices, small constant tensors."""
    npad = nsuper * SUPER
    cand = nsuper * 16
    candq = 2 * cand
    n = feats.shape[0]
    m = y_pred.shape[0] // ncores

    fpad = np.zeros((npad, D), np.float32)
    fpad[:n] = feats
    fT = np.ascontiguousarray(fpad.T)                  # [D, npad]
    fhi, flo = _split_bf16(fT)
    fhi = np.ascontiguousarray(fhi.reshape(4, 128, npad))
    flo = np.ascontiguousarray(flo.reshape(4, 128, npad))

    ypad = np.zeros((npad, 1), np.int32)
    ypad[:n, 0] = np.asarray(y, np.int64).astype(np.int32)

    # corr[p, s*16+u] = s*2048 + (1024 if p >= 64)
    srange = np.arange(nsuper, dtype=np.uint32) * SUPER
    corr = np.repeat(srange, 16)[None, :].repeat(128, axis=0)
    corr[m:, :] += 1024
    corr = np.ascontiguousarray(corr.astype(np.uint32))

    qbase = (np.arange(m, dtype=np.uint32) * candq)[:, None].repeat(TOPK, 1)
    qbase = np.ascontiguousarray(qbase)
    iotac = np.arange(C, dtype=np.float32)[None, :].repeat(m, 0)
    iotac = np.ascontiguousarray(iotac)

    in_maps = []
    for cid in range(ncores):
        yp = y_pred[cid * m:(cid + 1) * m]             # [m, D]
        ypT = np.ascontiguousarray(yp.T)               # [D, m]
        yhi, ylo = _split_bf16(ypT)
        in_maps.append({
            "fhi": fhi, "flo": flo,
            "yphi": np.ascontiguousarray(yhi.reshape(4, 128, m)),
            "yplo": np.ascontiguousarray(ylo.reshape(4, 128, m)),
            "ylab": ypad, "corr": corr, "qbase": qbase, "iotac": iotac,
        })
    return in_maps


def _run(y_pred, feats, y, nsuper=NSUPER_FULL, ncores=NCORES, trace=False):
    from concourse.bass_utils import run_bass_kernel_spmd

    nc = _build(nsuper, m=y_pred.shape[0] // ncores)
    in_maps = _prep_inputs(y_pred, feats, y, nsuper, ncores)
    res = run_bass_kernel_spmd(nc, in_maps, core_ids=list(range(ncores)),
                               trace=trace)
    outs = [r["out"] for r in res.results]
    return np.concatenate(outs, axis=0), res


def kernel(y_pred, image_features, y):
    y_pred = np.asarray(y_pred, np.float32)
    image_features = np.asarray(image_features, np.float32)
    out, _ = _run(y_pred, image_features, y)
    return out
